# revision 33
# baseline (speedup 1.0000x reference)
"""Trainium2 Bass kernel: causal self-attention with ALiBi.

Problem: B=4, S=2048, E=1024, H=16, D=64 (fp32 in/out).

Sharding (8 cores): core c -> (batch b = c//2, head-group g = c%2), 8 heads
per group. Each core computes QKV projection for its batch restricted to its
heads, attention for those (b, h), and a partial output projection over its
heads' dims. Host sums the two partials per batch and adds b_out.

On-chip layout (all matmuls bf16 inputs, fp32 PSUM accumulate):
 - QK^T proj (transposed outputs): lhsT = W chunks, rhs = x^T tiles ->
   psum [128, 512] where the 128 partitions are a head PAIR's q-dims (or
   k-dims).  Q stored as pair tiles [128, 4, 2048]; K stored zero-extended
   per head slot [128, 8, 2048] (even slot: data in partitions 0-63, zeros
   in 64-127; odd slot reversed) so a score matmul contracts over 128
   partitions with the other head's lanes killed by zero weights.
 - V in natural orientation [s, d] via lhsT = x^T chunks, rhs = Wv, stored
   [128, 16, 8, 65] with a ones column (65th) so the AV matmul also
   accumulates the softmax denominator l = sum_k P as output row 64.
 - scores^T tile [k=128, q=512] = (K chunk).T @ Q; P = exp(scores) * EM
   where EM is a host-precomputed slice-indexed [128, 2432] table per head
   carrying exp(slope*(k_abs-q_abs)) * causal_mask (ALiBi decay + mask).
 - AV: psum [65, 512] accumulates (V|1).T @ P over k chunks; row 64 = l.
 - attn^T = num^T * (1/l) (DVE, with 1/l broadcast across partitions via
   SBUF->SBUF DMA), cast bf16, laid out [128, 4, 2048] pair-major -> used
   directly as lhsT of the output projection.
"""

import sys

if '/opt/trn_rl_repo' not in sys.path:
    sys.path.insert(0, '/opt/trn_rl_repo')

import numpy as np
import ml_dtypes

import concourse.bass as bass
import concourse.tile as tile
from concourse import bacc, mybir
from concourse import bass_utils

BF16 = mybir.dt.bfloat16
F32 = mybir.dt.float32
AF = mybir.ActivationFunctionType

B, S, E, H, D = 4, 2048, 1024, 16, 64
NH = 8          # heads per core
N_CORES = 8
PAIRS = NH // 2
P = 128
ST = 512        # s/q tile (free dim)
NST = S // ST   # 4
NSC = S // P    # 16 s-chunks
NCC = E // P    # 8 contraction chunks
EMW = 2048      # EM table width (q-clipped tiles): 1536 + 512
EMOFF = 0

# ALiBi window: keys further than MARGIN/slope below the diagonal contribute
# exp(alibi) < e^-MARGIN ~ 2e-15 of the max element and are skipped at chunk
# granularity.  Head h's slope is 2^-((h+1)/2).  Heads are assigned to the two
# core groups by descending window so both groups' slot s have similar cost;
# the per-slot chunk range is the UNION of the two groups' windows (the
# program is shared by all cores; EM zeros make padded chunks no-ops).
MARGIN = 20.0

SLOT_HEADS_A = [15, 13, 11, 9, 7, 5, 3, 1]
SLOT_HEADS_B = [14, 12, 10, 8, 6, 4, 2, 0]


def _jlo(h, i):
    import math as _m
    delta = MARGIN / (2.0 ** (-(h + 1) / 2.0))
    return max(0, _m.ceil((ST * i - delta - (P - 1)) / P))


def k_chunks_for(s, i):
    lo = min(_jlo(SLOT_HEADS_A[s], i), _jlo(SLOT_HEADS_B[s], i))
    return list(range(lo, 4 * i + 4))


def k_chunks_for_pair(pr, i):
    lo = min(_jlo(h, i) for s in (2 * pr, 2 * pr + 1)
             for h in (SLOT_HEADS_A[s], SLOT_HEADS_B[s]))
    return list(range(lo, 4 * i + 4))


def _slopes():
    x = (2.0 ** 8) ** (1.0 / H)
    return np.array([1.0 / x ** (i + 1) for i in range(H)], dtype=np.float64)


def _build_em(heads):
    """EM[p, s, c] = exp(slope_h * m) * [m <= 0], m = k - c + EMOFF  (h = heads[s])."""
    slopes = _slopes()
    k = np.arange(P, dtype=np.float64)[:, None]
    c = np.arange(EMW, dtype=np.float64)[None, :]
    m = k - (c + 384) + 384  # EM'[k,c] = EM_old[k, c+384]; a0 shifted by -384
    em = np.zeros((P, NH, EMW), dtype=np.float32)
    for s, h in enumerate(heads):
        v = np.exp(slopes[h] * m) * (m <= 0)
        em[:, s, :] = v.astype(np.float32)
    return em.astype(ml_dtypes.bfloat16)


_PROG_CACHE = {}


def _build_program():
    if 'prog' in _PROG_CACHE:
        return _PROG_CACHE['prog']

    nc = bacc.Bacc("TRN2", target_bir_lowering=False, debug=False,
                   num_devices=N_CORES)

    xT = nc.dram_tensor("xT", [NST, P, NCC, ST], BF16, kind="ExternalInput")
    wqk = nc.dram_tensor("wqk", [P, NCC, 2 * NH * D], BF16, kind="ExternalInput")
    wv = nc.dram_tensor("wv", [P, NCC, NH * D], BF16, kind="ExternalInput")
    wout = nc.dram_tensor("wout", [P, PAIRS, E], BF16, kind="ExternalInput")
    bqk = nc.dram_tensor("bqk", [P, NCC], F32, kind="ExternalInput")
    bv = nc.dram_tensor("bv", [NH * D], BF16, kind="ExternalInput")
    em = nc.dram_tensor("em", [P, NH, EMW], BF16, kind="ExternalInput")
    out = nc.dram_tensor("out", [S, E], F32, kind="ExternalOutput")

    with tile.TileContext(nc) as tc:
        with (
            tc.tile_pool(name="persist", bufs=1) as pp,
            tc.tile_pool(name="xt", bufs=2) as xtp,
            tc.tile_pool(name="pm", bufs=3) as pmp,
            tc.tile_pool(name="misc", bufs=3) as misc,
            tc.tile_pool(name="osb", bufs=3) as osb,
            tc.tile_pool(name="drp", bufs=8, space="DRAM") as drp,
        ):
            wqk_sb = pp.tile([P, NCC, 2 * NH * D], BF16, tag="wqk")
            nc.sync.dma_start(wqk_sb[:, 0:NCC // 2], wqk.ap()[:, 0:NCC // 2])
            nc.gpsimd.dma_start(wqk_sb[:, NCC // 2:], wqk.ap()[:, NCC // 2:])
            wv_sb = pp.tile([P, NCC, NH * D], BF16, tag="wv")
            nc.gpsimd.dma_start(wv_sb, wv.ap())
            wout_sb = pp.tile([P, PAIRS, E], BF16, tag="wout")
            nc.gpsimd.dma_start(wout_sb, wout.ap())
            em_sb = pp.tile([P, NH, EMW], BF16, tag="em")
            nc.gpsimd.dma_start(em_sb, em.ap())
            bqk_sb = pp.tile([P, NCC], F32, tag="bqk")
            nc.sync.dma_start(bqk_sb, bqk.ap())
            bv_bc = pp.tile([P, NH * D], BF16, tag="bvbc")
            nc.sync.dma_start(bv_bc, bv.ap()[None, :].to_broadcast([P, NH * D]))

            q_sb = pp.tile([P, PAIRS, S], BF16, tag="q")
            k_ext = pp.tile([P, NH, S], BF16, tag="k")
            v_sb = pp.tile([P, NSC, NH, D + 1], BF16, tag="v")
            attn_sb = pp.tile([P, PAIRS, S], BF16, tag="attn")

            nc.gpsimd.memset(k_ext, 0.0)
            nc.vector.memset(v_sb[:, :, :, D:D + 1], 1.0)

            # ---- Phase 1: QKV projections ----
            p1 = tc.tile_pool(name="qkvps", bufs=2, space="PSUM")
            qkps = vps = p1.__enter__()
            # ---- Phase 2: attention (i-outer; outproj interleaved per i) ----
            p2a = tc.tile_pool(name="scps", bufs=2, space="PSUM")
            scps = p2a.__enter__()
            p2b = tc.tile_pool(name="avps", bufs=1, space="PSUM")
            avps = p2b.__enter__()
            for i in range(NST):
                qsl = slice(i * ST, (i + 1) * ST)
                sl = slice(i * ST, (i + 1) * ST)
                xt = xtp.tile([P, NCC, ST], BF16, tag="xt")
                nc.scalar.dma_start(xt, xT.ap()[i])
                for m in range(2 * PAIRS * 2 // 2):  # 8 e' chunks
                    ps = qkps.tile([P, ST], F32, tag="ps")
                    for c in range(NCC):
                        nc.tensor.matmul(
                            ps, lhsT=wqk_sb[:, c, m * P:(m + 1) * P],
                            rhs=xt[:, c, :], start=(c == 0), stop=(c == NCC - 1))
                    pr = m // 2
                    if m % 2 == 0:  # Q pair
                        nc.scalar.activation(q_sb[:, pr, sl], ps, AF.Identity,
                                             bias=bqk_sb[:, m:m + 1])
                    else:  # K pair -> zero-extended slots
                        nc.scalar.activation(k_ext[0:D, 2 * pr, sl], ps[0:D],
                                             AF.Identity, bias=bqk_sb[0:D, m:m + 1])
                        nc.scalar.activation(k_ext[D:P, 2 * pr + 1, sl], ps[D:P],
                                             AF.Identity, bias=bqk_sb[D:P, m:m + 1])
                # V projection for this s-tile's 4 s-chunks
                for s4 in range(4):
                    sc = 4 * i + s4
                    vp = vps.tile([P, NH * D], F32, tag="ps")
                    for c in range(NCC):
                        nc.tensor.matmul(
                            vp, lhsT=xt[:, c, s4 * P:(s4 + 1) * P],
                            rhs=wv_sb[:, c, :], start=(c == 0), stop=(c == NCC - 1))
                    nc.vector.tensor_add(
                        out=v_sb[:, sc, :, 0:D],
                        in0=vp.rearrange("p (h d) -> p h d", h=NH),
                        in1=bv_bc.rearrange("p (h d) -> p h d", h=NH))
                for pr in range(PAIRS):
                    s0, s1 = 2 * pr, 2 * pr + 1
                    av0 = avps.tile([P, ST], F32, tag="av0")
                    av1 = avps.tile([P, ST], F32, tag="av1")
                    js = k_chunks_for_pair(pr, i)
                    for jj, j in enumerate(js):
                        # q columns below the diagonal of this k-chunk are
                        # fully masked; clip them (qlo is 0 for jj == 0).
                        qlo = max(0, P * j - ST * i)
                        w = ST - qlo
                        qs = slice(i * ST + qlo, (i + 1) * ST)
                        scp = scps.tile([P, 2 * ST], F32, tag="scp")
                        scp3 = scp.rearrange("p (t st) -> p t st", t=2)
                        nc.tensor.matmul(
                            scp[:, 0:w], lhsT=k_ext[:, s0, j * P:(j + 1) * P],
                            rhs=q_sb[:, pr, qs], start=True, stop=True)
                        nc.tensor.matmul(
                            scp[:, ST:ST + w], lhsT=k_ext[:, s1, j * P:(j + 1) * P],
                            rhs=q_sb[:, pr, qs], start=True, stop=True)
                        pm = pmp.tile([P, 2 * ST], BF16, tag="pm")
                        pm3 = pm.rearrange("p (t st) -> p t st", t=2)
                        nc.scalar.activation(pm3[:, :, 0:w], scp3[:, :, 0:w],
                                             AF.Exp)
                        pm2 = pmp.tile([P, 2 * ST], BF16, tag="pm2")
                        pm23 = pm2.rearrange("p (t st) -> p t st", t=2)
                        a0 = EMOFF - P * j + ST * i + qlo
                        nc.vector.tensor_mul(
                            pm23[:, :, 0:w], pm3[:, :, 0:w],
                            em_sb[:, s0:s0 + 2, a0:a0 + w])
                        nc.tensor.matmul(
                            av0[0:D + 1, qlo:ST], lhsT=v_sb[:, j, s0, :],
                            rhs=pm2[:, 0:w],
                            start=(jj == 0), stop=(jj == len(js) - 1))
                        nc.tensor.matmul(
                            av1[0:D + 1, qlo:ST], lhsT=v_sb[:, j, s1, :],
                            rhs=pm2[:, ST:ST + w],
                            start=(jj == 0), stop=(jj == len(js) - 1))
                    for odd, av in ((0, av0), (1, av1)):
                        lrow = misc.tile([D + 1, ST], F32, tag="lrow")
                        nc.vector.tensor_copy(lrow[D:D + 1, :], av[D:D + 1, :])
                        avs = misc.tile([D, ST], F32, tag="avs")
                        nc.scalar.activation(avs, av[0:D, :], AF.Copy)
                        scratch = drp.tile([1, ST], F32, tag="lscr")
                        nc.sync.dma_start(scratch, lrow[D:D + 1, :])
                        l_bc = misc.tile([D, ST], F32, tag="lbc")
                        nc.sync.dma_start(
                            l_bc,
                            bass.AP(tensor=scratch.tensor, offset=scratch.offset,
                                    ap=[[0, D]] + scratch.ap[1:]))
                        linv_bc = misc.tile([D, ST], F32, tag="linvbc")
                        nc.vector.reciprocal_approx_fast(out=linv_bc, in_=l_bc)
                        if not odd:
                            nc.vector.tensor_mul(attn_sb[0:D, pr, qsl],
                                                 avs, linv_bc)
                        else:
                            atmp = misc.tile([D, ST], BF16, tag="atmp")
                            nc.vector.tensor_mul(atmp, avs, linv_bc)
                            nc.sync.dma_start(attn_sb[D:P, pr, qsl], atmp)
                # output projection for this q-tile (reuses qkv psum banks)
                for qc in range(4 * i, 4 * i + 4):
                    for n in range(E // ST):
                        op = qkps.tile([P, ST], F32, tag="ps")
                        for p_ in range(PAIRS):
                            nc.tensor.matmul(
                                op, lhsT=attn_sb[:, p_, qc * P:(qc + 1) * P],
                                rhs=wout_sb[:, p_, n * ST:(n + 1) * ST],
                                start=(p_ == 0), stop=(p_ == PAIRS - 1))
                        ot = osb.tile([P, ST], F32, tag="ot")
                        nc.scalar.activation(ot, op, AF.Copy)
                        nc.sync.dma_start(
                            out.ap()[qc * P:(qc + 1) * P, n * ST:(n + 1) * ST], ot)
            p2b.__exit__(None, None, None)
            p2a.__exit__(None, None, None)
            p1.__exit__(None, None, None)

    nc.compile()
    _PROG_CACHE['prog'] = nc
    return nc


def _head_groups():
    return [SLOT_HEADS_A, SLOT_HEADS_B]


def _prep_core_inputs(x, W_qkv, b_qkv, W_out, b_out):
    """Build the 8 per-core input dicts (host-side shard + transform)."""
    groups = _head_groups()
    bf = ml_dtypes.bfloat16
    per_group = []
    for heads in groups:
        qcols = []
        kcols = []
        bqk_l = []
        for p in range(PAIRS):
            h0, h1 = heads[2 * p], heads[2 * p + 1]
            wq = np.concatenate([W_qkv[:, h0 * D:(h0 + 1) * D],
                                 W_qkv[:, h1 * D:(h1 + 1) * D]], axis=1) / 8.0
            wk = np.concatenate([W_qkv[:, E + h0 * D:E + (h0 + 1) * D],
                                 W_qkv[:, E + h1 * D:E + (h1 + 1) * D]], axis=1)
            qcols.append(wq)
            kcols.append(wk)
            bqk_l.append(np.concatenate([b_qkv[h0 * D:(h0 + 1) * D],
                                         b_qkv[h1 * D:(h1 + 1) * D]]) / 8.0)
            bqk_l.append(np.concatenate([b_qkv[E + h0 * D:E + (h0 + 1) * D],
                                         b_qkv[E + h1 * D:E + (h1 + 1) * D]]))
        # interleave Q-pair / K-pair chunks: m even = Q, m odd = K
        wqk_l = np.empty((E, 2 * NH * D), dtype=np.float32)
        for p in range(PAIRS):
            wqk_l[:, (2 * p) * P:(2 * p + 1) * P] = qcols[p]
            wqk_l[:, (2 * p + 1) * P:(2 * p + 2) * P] = kcols[p]
        bqk_full = np.empty(2 * NH * D, dtype=np.float32)
        for m in range(2 * PAIRS):
            bqk_full[m * P:(m + 1) * P] = bqk_l[m]
        wv_l = np.concatenate(
            [W_qkv[:, 2 * E + h * D:2 * E + (h + 1) * D] for h in heads], axis=1)
        bv_l = np.concatenate(
            [b_qkv[2 * E + h * D:2 * E + (h + 1) * D] for h in heads])
        wout_l = np.concatenate([W_out[h * D:(h + 1) * D, :] for h in heads],
                                axis=0)
        # pre-tile into the exact SBUF layouts for large-descriptor DMAs
        wqk_t = np.ascontiguousarray(
            wqk_l.reshape(NCC, P, 2 * NH * D).transpose(1, 0, 2)).astype(bf)
        wv_t = np.ascontiguousarray(
            wv_l.reshape(NCC, P, NH * D).transpose(1, 0, 2)).astype(bf)
        wout_t = np.ascontiguousarray(
            wout_l.reshape(PAIRS, P, E).transpose(1, 0, 2)).astype(bf)
        bqk_t = np.ascontiguousarray(
            bqk_full.reshape(NCC, P).T).astype(np.float32)
        per_group.append(dict(
            wqk=wqk_t, wv=wv_t, wout=wout_t,
            bqk=bqk_t, bv=bv_l.astype(bf),
            em=_build_em(heads)))

    in_maps = []
    xt_cache = {}
    for c in range(N_CORES):
        b, g = c // 2, c % 2
        m = dict(per_group[g])
        if b not in xt_cache:
            xt = x[b].T  # [E, S]
            xt_cache[b] = np.ascontiguousarray(
                xt.reshape(NCC, P, NST, ST).transpose(2, 1, 0, 3)).astype(bf)
        m['xT'] = xt_cache[b]
        in_maps.append(m)
    return in_maps


def _run(inputs, trace=False, tmpdir=None, trace_cores=None):
    x = np.asarray(inputs['x'], dtype=np.float32)
    W_qkv = np.asarray(inputs['W_qkv'], dtype=np.float32)
    b_qkv = np.asarray(inputs['b_qkv'], dtype=np.float32)
    W_out = np.asarray(inputs['W_out'], dtype=np.float32)
    b_out = np.asarray(inputs['b_out'], dtype=np.float32)

    nc = _build_program()
    in_maps = _prep_core_inputs(x, W_qkv, b_qkv, W_out, b_out)
    res = bass_utils.run_bass_kernel_spmd(
        nc, in_maps, core_ids=list(range(N_CORES)), trace=trace, tmpdir=tmpdir,
        trace_cores=trace_cores)
    out = np.empty((B, S, E), dtype=np.float32)
    for b in range(B):
        out[b] = res.results[2 * b]['out'] + res.results[2 * b + 1]['out'] + b_out
    return out, res


def kernel(**inputs) -> np.ndarray:
    out, _ = _run(inputs)
    return out


# revision 34
# speedup vs baseline: 1.1499x; 1.1499x over previous
"""Trainium2 Bass kernel: causal self-attention with ALiBi.

Problem: B=4, S=2048, E=1024, H=16, D=64 (fp32 in/out).

Sharding (8 cores): core c -> (batch b = c//2, head-group g = c%2), 8 heads
per group. Each core computes QKV projection for its batch restricted to its
heads, attention for those (b, h), and a partial output projection over its
heads' dims. Host sums the two partials per batch and adds b_out.

On-chip layout (all matmuls bf16 inputs, fp32 PSUM accumulate):
 - QK^T proj (transposed outputs): lhsT = W chunks, rhs = x^T tiles ->
   psum [128, 512] where the 128 partitions are a head PAIR's q-dims (or
   k-dims).  Q stored as pair tiles [128, 4, 2048]; K stored zero-extended
   per head slot [128, 8, 2048] (even slot: data in partitions 0-63, zeros
   in 64-127; odd slot reversed) so a score matmul contracts over 128
   partitions with the other head's lanes killed by zero weights.
 - V in natural orientation [s, d] via lhsT = x^T chunks, rhs = Wv, stored
   [128, 16, 8, 65] with a ones column (65th) so the AV matmul also
   accumulates the softmax denominator l = sum_k P as output row 64.
 - scores^T tile [k=128, q=512] = (K chunk).T @ Q; P = exp(scores) * EM
   where EM is a host-precomputed slice-indexed [128, 2432] table per head
   carrying exp(slope*(k_abs-q_abs)) * causal_mask (ALiBi decay + mask).
 - AV: psum [65, 512] accumulates (V|1).T @ P over k chunks; row 64 = l.
 - attn^T = num^T * (1/l) (DVE, with 1/l broadcast across partitions via
   SBUF->SBUF DMA), cast bf16, laid out [128, 4, 2048] pair-major -> used
   directly as lhsT of the output projection.
"""

import sys

if '/opt/trn_rl_repo' not in sys.path:
    sys.path.insert(0, '/opt/trn_rl_repo')

import numpy as np
import ml_dtypes

import concourse.bass as bass
import concourse.tile as tile
from concourse import bacc, mybir
from concourse import bass_utils

BF16 = mybir.dt.bfloat16
F32 = mybir.dt.float32
AF = mybir.ActivationFunctionType

B, S, E, H, D = 4, 2048, 1024, 16, 64
NH = 8          # heads per core
N_CORES = 8
PAIRS = NH // 2
P = 128
ST = 512        # s/q tile (free dim)
NST = S // ST   # 4
NSC = S // P    # 16 s-chunks
NCC = E // P    # 8 contraction chunks
EMW = 2048      # EM table width (q-clipped tiles): 1536 + 512
EMOFF = 0

# ALiBi window: keys further than MARGIN/slope below the diagonal contribute
# exp(alibi) < e^-MARGIN ~ 2e-15 of the max element and are skipped at chunk
# granularity.  Head h's slope is 2^-((h+1)/2).  Heads are assigned to the two
# core groups by descending window so both groups' slot s have similar cost;
# the per-slot chunk range is the UNION of the two groups' windows (the
# program is shared by all cores; EM zeros make padded chunks no-ops).
MARGIN = 20.0

SLOT_HEADS_A = [15, 13, 11, 9, 7, 5, 3, 1]
SLOT_HEADS_B = [14, 12, 10, 8, 6, 4, 2, 0]


def _jlo(h, i):
    import math as _m
    delta = MARGIN / (2.0 ** (-(h + 1) / 2.0))
    return max(0, _m.ceil((ST * i - delta - (P - 1)) / P))


def k_chunks_for(s, i):
    lo = min(_jlo(SLOT_HEADS_A[s], i), _jlo(SLOT_HEADS_B[s], i))
    return list(range(lo, 4 * i + 4))


def k_chunks_for_pair(pr, i):
    lo = min(_jlo(h, i) for s in (2 * pr, 2 * pr + 1)
             for h in (SLOT_HEADS_A[s], SLOT_HEADS_B[s]))
    return list(range(lo, 4 * i + 4))


def _slopes():
    x = (2.0 ** 8) ** (1.0 / H)
    return np.array([1.0 / x ** (i + 1) for i in range(H)], dtype=np.float64)


def _build_em(heads):
    """EM[p, s, c] = exp(slope_h * m) * [m <= 0], m = k - c + EMOFF  (h = heads[s])."""
    slopes = _slopes()
    k = np.arange(P, dtype=np.float64)[:, None]
    c = np.arange(EMW, dtype=np.float64)[None, :]
    m = k - (c + 384) + 384  # EM'[k,c] = EM_old[k, c+384]; a0 shifted by -384
    em = np.zeros((P, NH, EMW), dtype=np.float32)
    for s, h in enumerate(heads):
        v = np.exp(slopes[h] * m) * (m <= 0)
        em[:, s, :] = v.astype(np.float32)
    return em.astype(ml_dtypes.bfloat16)


_PROG_CACHE = {}


def _build_program():
    if 'prog' in _PROG_CACHE:
        return _PROG_CACHE['prog']

    nc = bacc.Bacc("TRN2", target_bir_lowering=False, debug=False,
                   num_devices=N_CORES)

    xT = nc.dram_tensor("xT", [NST, P, NCC, ST], BF16, kind="ExternalInput")
    wqk = nc.dram_tensor("wqk", [P, NCC, 2 * NH * D], BF16, kind="ExternalInput")
    wv = nc.dram_tensor("wv", [P, NCC, NH * D], BF16, kind="ExternalInput")
    wout = nc.dram_tensor("wout", [P, PAIRS, E], BF16, kind="ExternalInput")
    bqk = nc.dram_tensor("bqk", [P, NCC], F32, kind="ExternalInput")
    bv = nc.dram_tensor("bv", [NH * D], BF16, kind="ExternalInput")
    em = nc.dram_tensor("em", [P, NH, EMW], BF16, kind="ExternalInput")
    out = nc.dram_tensor("out", [S, E], F32, kind="ExternalOutput")

    with tile.TileContext(nc) as tc:
        with (
            tc.tile_pool(name="persist", bufs=1) as pp,
            tc.tile_pool(name="xt", bufs=2) as xtp,
            tc.tile_pool(name="pm", bufs=3) as pmp,
            tc.tile_pool(name="misc", bufs=3) as misc,
            tc.tile_pool(name="osb", bufs=3) as osb,
            tc.tile_pool(name="drp", bufs=8, space="DRAM") as drp,
        ):
            wqk_sb = pp.tile([P, NCC, 2 * NH * D], BF16, tag="wqk")
            nc.sync.dma_start(wqk_sb[:, 0:NCC // 2], wqk.ap()[:, 0:NCC // 2])
            nc.gpsimd.dma_start(wqk_sb[:, NCC // 2:], wqk.ap()[:, NCC // 2:])
            wv_sb = pp.tile([P, NCC, NH * D], BF16, tag="wv")
            nc.gpsimd.dma_start(wv_sb, wv.ap())
            wout_sb = pp.tile([P, PAIRS, E], BF16, tag="wout")
            nc.gpsimd.dma_start(wout_sb, wout.ap())
            em_sb = pp.tile([P, NH, EMW], BF16, tag="em")
            nc.gpsimd.dma_start(em_sb, em.ap())
            bqk_sb = pp.tile([P, NCC], F32, tag="bqk")
            nc.sync.dma_start(bqk_sb, bqk.ap())
            bv_bc = pp.tile([P, NH * D], BF16, tag="bvbc")
            nc.sync.dma_start(bv_bc, bv.ap()[None, :].to_broadcast([P, NH * D]))

            q_sb = pp.tile([P, PAIRS, S], BF16, tag="q")
            k_ext = pp.tile([P, NH, S], BF16, tag="k")
            v_sb = pp.tile([P, NSC, NH, D + 1], BF16, tag="v")
            attn_sb = pp.tile([P, PAIRS, S], BF16, tag="attn")

            nc.gpsimd.memset(k_ext, 0.0)
            nc.vector.memset(v_sb[:, :, :, D:D + 1], 1.0)

            # ---- Phase 1: QKV projections ----
            p1 = tc.tile_pool(name="qkvps", bufs=2, space="PSUM")
            qkps = vps = p1.__enter__()
            for i in range(NST):
                sl = slice(i * ST, (i + 1) * ST)
                xt = xtp.tile([P, NCC, ST], BF16, tag="xt")
                nc.scalar.dma_start(xt, xT.ap()[i])
                for m in range(2 * PAIRS * 2 // 2):  # 8 e' chunks
                    ps = qkps.tile([P, ST], F32, tag="ps")
                    for c in range(NCC):
                        nc.tensor.matmul(
                            ps, lhsT=wqk_sb[:, c, m * P:(m + 1) * P],
                            rhs=xt[:, c, :], start=(c == 0), stop=(c == NCC - 1))
                    pr = m // 2
                    if m % 2 == 0:  # Q pair
                        nc.scalar.activation(q_sb[:, pr, sl], ps, AF.Identity,
                                             bias=bqk_sb[:, m:m + 1])
                    else:  # K pair -> zero-extended slots
                        nc.scalar.activation(k_ext[0:D, 2 * pr, sl], ps[0:D],
                                             AF.Identity, bias=bqk_sb[0:D, m:m + 1])
                        nc.scalar.activation(k_ext[D:P, 2 * pr + 1, sl], ps[D:P],
                                             AF.Identity, bias=bqk_sb[D:P, m:m + 1])
                # V projection for this s-tile's 4 s-chunks
                for s4 in range(4):
                    sc = 4 * i + s4
                    vp = vps.tile([P, NH * D], F32, tag="ps")
                    for c in range(NCC):
                        nc.tensor.matmul(
                            vp, lhsT=xt[:, c, s4 * P:(s4 + 1) * P],
                            rhs=wv_sb[:, c, :], start=(c == 0), stop=(c == NCC - 1))
                    nc.vector.tensor_add(
                        out=v_sb[:, sc, :, 0:D],
                        in0=vp.rearrange("p (h d) -> p h d", h=NH),
                        in1=bv_bc.rearrange("p (h d) -> p h d", h=NH))

            # ---- Phase 2: attention (i-outer; outproj interleaved per i) ----
            p2a = tc.tile_pool(name="scps", bufs=2, space="PSUM")
            scps = p2a.__enter__()
            p2b = tc.tile_pool(name="avps", bufs=1, space="PSUM")
            avps = p2b.__enter__()
            for i in range(NST):
                qsl = slice(i * ST, (i + 1) * ST)
                for pr in range(PAIRS):
                    s0, s1 = 2 * pr, 2 * pr + 1
                    av0 = avps.tile([P, ST], F32, tag="av0")
                    av1 = avps.tile([P, ST], F32, tag="av1")
                    js = k_chunks_for_pair(pr, i)
                    for jj, j in enumerate(js):
                        # q columns below the diagonal of this k-chunk are
                        # fully masked; clip them (qlo is 0 for jj == 0).
                        qlo = max(0, P * j - ST * i)
                        w = ST - qlo
                        qs = slice(i * ST + qlo, (i + 1) * ST)
                        scp = scps.tile([P, 2 * ST], F32, tag="scp")
                        scp3 = scp.rearrange("p (t st) -> p t st", t=2)
                        nc.tensor.matmul(
                            scp[:, 0:w], lhsT=k_ext[:, s0, j * P:(j + 1) * P],
                            rhs=q_sb[:, pr, qs], start=True, stop=True)
                        nc.tensor.matmul(
                            scp[:, ST:ST + w], lhsT=k_ext[:, s1, j * P:(j + 1) * P],
                            rhs=q_sb[:, pr, qs], start=True, stop=True)
                        pm = pmp.tile([P, 2 * ST], BF16, tag="pm")
                        pm3 = pm.rearrange("p (t st) -> p t st", t=2)
                        nc.scalar.activation(pm3[:, :, 0:w], scp3[:, :, 0:w],
                                             AF.Exp)
                        pm2 = pmp.tile([P, 2 * ST], BF16, tag="pm2")
                        pm23 = pm2.rearrange("p (t st) -> p t st", t=2)
                        a0 = EMOFF - P * j + ST * i + qlo
                        nc.vector.tensor_mul(
                            pm23[:, :, 0:w], pm3[:, :, 0:w],
                            em_sb[:, s0:s0 + 2, a0:a0 + w])
                        nc.tensor.matmul(
                            av0[0:D + 1, qlo:ST], lhsT=v_sb[:, j, s0, :],
                            rhs=pm2[:, 0:w],
                            start=(jj == 0), stop=(jj == len(js) - 1))
                        nc.tensor.matmul(
                            av1[0:D + 1, qlo:ST], lhsT=v_sb[:, j, s1, :],
                            rhs=pm2[:, ST:ST + w],
                            start=(jj == 0), stop=(jj == len(js) - 1))
                    for odd, av in ((0, av0), (1, av1)):
                        lrow = misc.tile([D + 1, ST], F32, tag="lrow")
                        nc.vector.tensor_copy(lrow[D:D + 1, :], av[D:D + 1, :])
                        avs = misc.tile([D, ST], F32, tag="avs")
                        nc.scalar.activation(avs, av[0:D, :], AF.Copy)
                        scratch = drp.tile([1, ST], F32, tag="lscr")
                        nc.sync.dma_start(scratch, lrow[D:D + 1, :])
                        l_bc = misc.tile([D, ST], F32, tag="lbc")
                        nc.sync.dma_start(
                            l_bc,
                            bass.AP(tensor=scratch.tensor, offset=scratch.offset,
                                    ap=[[0, D]] + scratch.ap[1:]))
                        linv_bc = misc.tile([D, ST], F32, tag="linvbc")
                        nc.vector.reciprocal_approx_fast(out=linv_bc, in_=l_bc)
                        if not odd:
                            nc.vector.tensor_mul(attn_sb[0:D, pr, qsl],
                                                 avs, linv_bc)
                        else:
                            atmp = misc.tile([D, ST], BF16, tag="atmp")
                            nc.vector.tensor_mul(atmp, avs, linv_bc)
                            nc.sync.dma_start(attn_sb[D:P, pr, qsl], atmp)
                # output projection for this q-tile (reuses qkv psum banks)
                for qc in range(4 * i, 4 * i + 4):
                    for n in range(E // ST):
                        op = qkps.tile([P, ST], F32, tag="ps")
                        for p_ in range(PAIRS):
                            nc.tensor.matmul(
                                op, lhsT=attn_sb[:, p_, qc * P:(qc + 1) * P],
                                rhs=wout_sb[:, p_, n * ST:(n + 1) * ST],
                                start=(p_ == 0), stop=(p_ == PAIRS - 1))
                        ot = osb.tile([P, ST], F32, tag="ot")
                        nc.scalar.activation(ot, op, AF.Copy)
                        nc.sync.dma_start(
                            out.ap()[qc * P:(qc + 1) * P, n * ST:(n + 1) * ST], ot)
            p2b.__exit__(None, None, None)
            p2a.__exit__(None, None, None)
            p1.__exit__(None, None, None)

    nc.compile()
    _PROG_CACHE['prog'] = nc
    return nc


def _head_groups():
    return [SLOT_HEADS_A, SLOT_HEADS_B]


def _prep_core_inputs(x, W_qkv, b_qkv, W_out, b_out):
    """Build the 8 per-core input dicts (host-side shard + transform)."""
    groups = _head_groups()
    bf = ml_dtypes.bfloat16
    per_group = []
    for heads in groups:
        qcols = []
        kcols = []
        bqk_l = []
        for p in range(PAIRS):
            h0, h1 = heads[2 * p], heads[2 * p + 1]
            wq = np.concatenate([W_qkv[:, h0 * D:(h0 + 1) * D],
                                 W_qkv[:, h1 * D:(h1 + 1) * D]], axis=1) / 8.0
            wk = np.concatenate([W_qkv[:, E + h0 * D:E + (h0 + 1) * D],
                                 W_qkv[:, E + h1 * D:E + (h1 + 1) * D]], axis=1)
            qcols.append(wq)
            kcols.append(wk)
            bqk_l.append(np.concatenate([b_qkv[h0 * D:(h0 + 1) * D],
                                         b_qkv[h1 * D:(h1 + 1) * D]]) / 8.0)
            bqk_l.append(np.concatenate([b_qkv[E + h0 * D:E + (h0 + 1) * D],
                                         b_qkv[E + h1 * D:E + (h1 + 1) * D]]))
        # interleave Q-pair / K-pair chunks: m even = Q, m odd = K
        wqk_l = np.empty((E, 2 * NH * D), dtype=np.float32)
        for p in range(PAIRS):
            wqk_l[:, (2 * p) * P:(2 * p + 1) * P] = qcols[p]
            wqk_l[:, (2 * p + 1) * P:(2 * p + 2) * P] = kcols[p]
        bqk_full = np.empty(2 * NH * D, dtype=np.float32)
        for m in range(2 * PAIRS):
            bqk_full[m * P:(m + 1) * P] = bqk_l[m]
        wv_l = np.concatenate(
            [W_qkv[:, 2 * E + h * D:2 * E + (h + 1) * D] for h in heads], axis=1)
        bv_l = np.concatenate(
            [b_qkv[2 * E + h * D:2 * E + (h + 1) * D] for h in heads])
        wout_l = np.concatenate([W_out[h * D:(h + 1) * D, :] for h in heads],
                                axis=0)
        # pre-tile into the exact SBUF layouts for large-descriptor DMAs
        wqk_t = np.ascontiguousarray(
            wqk_l.reshape(NCC, P, 2 * NH * D).transpose(1, 0, 2)).astype(bf)
        wv_t = np.ascontiguousarray(
            wv_l.reshape(NCC, P, NH * D).transpose(1, 0, 2)).astype(bf)
        wout_t = np.ascontiguousarray(
            wout_l.reshape(PAIRS, P, E).transpose(1, 0, 2)).astype(bf)
        bqk_t = np.ascontiguousarray(
            bqk_full.reshape(NCC, P).T).astype(np.float32)
        per_group.append(dict(
            wqk=wqk_t, wv=wv_t, wout=wout_t,
            bqk=bqk_t, bv=bv_l.astype(bf),
            em=_build_em(heads)))

    in_maps = []
    xt_cache = {}
    for c in range(N_CORES):
        b, g = c // 2, c % 2
        m = dict(per_group[g])
        if b not in xt_cache:
            xt = x[b].T  # [E, S]
            xt_cache[b] = np.ascontiguousarray(
                xt.reshape(NCC, P, NST, ST).transpose(2, 1, 0, 3)).astype(bf)
        m['xT'] = xt_cache[b]
        in_maps.append(m)
    return in_maps


def _run(inputs, trace=False, tmpdir=None, trace_cores=None):
    x = np.asarray(inputs['x'], dtype=np.float32)
    W_qkv = np.asarray(inputs['W_qkv'], dtype=np.float32)
    b_qkv = np.asarray(inputs['b_qkv'], dtype=np.float32)
    W_out = np.asarray(inputs['W_out'], dtype=np.float32)
    b_out = np.asarray(inputs['b_out'], dtype=np.float32)

    nc = _build_program()
    in_maps = _prep_core_inputs(x, W_qkv, b_qkv, W_out, b_out)
    res = bass_utils.run_bass_kernel_spmd(
        nc, in_maps, core_ids=list(range(N_CORES)), trace=trace, tmpdir=tmpdir,
        trace_cores=trace_cores)
    out = np.empty((B, S, E), dtype=np.float32)
    for b in range(B):
        out[b] = res.results[2 * b]['out'] + res.results[2 * b + 1]['out'] + b_out
    return out, res


def kernel(**inputs) -> np.ndarray:
    out, _ = _run(inputs)
    return out


# revision 35
# speedup vs baseline: 1.1679x; 1.0157x over previous
"""Trainium2 Bass kernel: causal self-attention with ALiBi.

Problem: B=4, S=2048, E=1024, H=16, D=64 (fp32 in/out).

Sharding (8 cores): core c -> (batch b = c//2, head-group g = c%2), 8 heads
per group. Each core computes QKV projection for its batch restricted to its
heads, attention for those (b, h), and a partial output projection over its
heads' dims. Host sums the two partials per batch and adds b_out.

On-chip layout (all matmuls bf16 inputs, fp32 PSUM accumulate):
 - QK^T proj (transposed outputs): lhsT = W chunks, rhs = x^T tiles ->
   psum [128, 512] where the 128 partitions are a head PAIR's q-dims (or
   k-dims).  Q stored as pair tiles [128, 4, 2048]; K stored zero-extended
   per head slot [128, 8, 2048] (even slot: data in partitions 0-63, zeros
   in 64-127; odd slot reversed) so a score matmul contracts over 128
   partitions with the other head's lanes killed by zero weights.
 - V in natural orientation [s, d] via lhsT = x^T chunks, rhs = Wv, stored
   [128, 16, 8, 65] with a ones column (65th) so the AV matmul also
   accumulates the softmax denominator l = sum_k P as output row 64.
 - scores^T tile [k=128, q=512] = (K chunk).T @ Q; P = exp(scores) * EM
   where EM is a host-precomputed slice-indexed [128, 2432] table per head
   carrying exp(slope*(k_abs-q_abs)) * causal_mask (ALiBi decay + mask).
 - AV: psum [65, 512] accumulates (V|1).T @ P over k chunks; row 64 = l.
 - attn^T = num^T * (1/l) (DVE, with 1/l broadcast across partitions via
   SBUF->SBUF DMA), cast bf16, laid out [128, 4, 2048] pair-major -> used
   directly as lhsT of the output projection.
"""

import sys

if '/opt/trn_rl_repo' not in sys.path:
    sys.path.insert(0, '/opt/trn_rl_repo')

import numpy as np
import ml_dtypes

import concourse.bass as bass
import concourse.tile as tile
from concourse import bacc, mybir
from concourse import bass_utils

BF16 = mybir.dt.bfloat16
F32 = mybir.dt.float32
AF = mybir.ActivationFunctionType

B, S, E, H, D = 4, 2048, 1024, 16, 64
NH = 8          # heads per core
N_CORES = 8
PAIRS = NH // 2
P = 128
ST = 512        # s/q tile (free dim)
NST = S // ST   # 4
NSC = S // P    # 16 s-chunks
NCC = E // P    # 8 contraction chunks
EMW = 2048      # EM table width (q-clipped tiles): 1536 + 512
EMOFF = 0

# ALiBi window: keys further than MARGIN/slope below the diagonal contribute
# exp(alibi) < e^-MARGIN ~ 2e-15 of the max element and are skipped at chunk
# granularity.  Head h's slope is 2^-((h+1)/2).  Heads are assigned to the two
# core groups by descending window so both groups' slot s have similar cost;
# the per-slot chunk range is the UNION of the two groups' windows (the
# program is shared by all cores; EM zeros make padded chunks no-ops).
MARGIN = 16.0

SLOT_HEADS_A = [15, 13, 11, 9, 7, 5, 3, 1]
SLOT_HEADS_B = [14, 12, 10, 8, 6, 4, 2, 0]


def _jlo(h, i):
    import math as _m
    delta = MARGIN / (2.0 ** (-(h + 1) / 2.0))
    return max(0, _m.ceil((ST * i - delta - (P - 1)) / P))


def k_chunks_for(s, i):
    lo = min(_jlo(SLOT_HEADS_A[s], i), _jlo(SLOT_HEADS_B[s], i))
    return list(range(lo, 4 * i + 4))


def k_chunks_for_pair(pr, i):
    lo = min(_jlo(h, i) for s in (2 * pr, 2 * pr + 1)
             for h in (SLOT_HEADS_A[s], SLOT_HEADS_B[s]))
    return list(range(lo, 4 * i + 4))


def _slopes():
    x = (2.0 ** 8) ** (1.0 / H)
    return np.array([1.0 / x ** (i + 1) for i in range(H)], dtype=np.float64)


def _build_em(heads):
    """EM[p, s, c] = exp(slope_h * m) * [m <= 0], m = k - c + EMOFF  (h = heads[s])."""
    slopes = _slopes()
    k = np.arange(P, dtype=np.float64)[:, None]
    c = np.arange(EMW, dtype=np.float64)[None, :]
    m = k - (c + 384) + 384  # EM'[k,c] = EM_old[k, c+384]; a0 shifted by -384
    em = np.zeros((P, NH, EMW), dtype=np.float32)
    for s, h in enumerate(heads):
        v = np.exp(slopes[h] * m) * (m <= 0)
        em[:, s, :] = v.astype(np.float32)
    return em.astype(ml_dtypes.bfloat16)


_PROG_CACHE = {}


def _build_program():
    if 'prog' in _PROG_CACHE:
        return _PROG_CACHE['prog']

    nc = bacc.Bacc("TRN2", target_bir_lowering=False, debug=False,
                   num_devices=N_CORES)

    xT = nc.dram_tensor("xT", [NST, P, NCC, ST], BF16, kind="ExternalInput")
    wqk = nc.dram_tensor("wqk", [P, NCC, 2 * NH * D], BF16, kind="ExternalInput")
    wv = nc.dram_tensor("wv", [P, NCC, NH * D], BF16, kind="ExternalInput")
    wout = nc.dram_tensor("wout", [P, PAIRS, E], BF16, kind="ExternalInput")
    bqk = nc.dram_tensor("bqk", [P, NCC], F32, kind="ExternalInput")
    bv = nc.dram_tensor("bv", [NH * D], BF16, kind="ExternalInput")
    em = nc.dram_tensor("em", [P, NH, EMW], BF16, kind="ExternalInput")
    out = nc.dram_tensor("out", [S, E], F32, kind="ExternalOutput")

    with tile.TileContext(nc) as tc:
        with (
            tc.tile_pool(name="persist", bufs=1) as pp,
            tc.tile_pool(name="xt", bufs=2) as xtp,
            tc.tile_pool(name="pm", bufs=3) as pmp,
            tc.tile_pool(name="misc", bufs=3) as misc,
            tc.tile_pool(name="osb", bufs=3) as osb,
            tc.tile_pool(name="drp", bufs=8, space="DRAM") as drp,
        ):
            wqk_sb = pp.tile([P, NCC, 2 * NH * D], BF16, tag="wqk")
            for _q in range(4):
                _sl = slice(_q * 2, _q * 2 + 2)
                _eng = nc.sync if _q % 2 == 0 else nc.gpsimd
                _eng.dma_start(wqk_sb[:, _sl], wqk.ap()[:, _sl])
            wv_sb = pp.tile([P, NCC, NH * D], BF16, tag="wv")
            nc.gpsimd.dma_start(wv_sb, wv.ap())
            wout_sb = pp.tile([P, PAIRS, E], BF16, tag="wout")
            nc.gpsimd.dma_start(wout_sb, wout.ap())
            em_sb = pp.tile([P, NH, EMW], BF16, tag="em")
            nc.gpsimd.dma_start(em_sb, em.ap())
            bqk_sb = pp.tile([P, NCC], F32, tag="bqk")
            nc.sync.dma_start(bqk_sb, bqk.ap())
            bv_bc = pp.tile([P, NH * D], BF16, tag="bvbc")
            nc.sync.dma_start(bv_bc, bv.ap()[None, :].to_broadcast([P, NH * D]))

            q_sb = pp.tile([P, PAIRS, S], BF16, tag="q")
            k_ext = pp.tile([P, NH, S], BF16, tag="k")
            v_sb = pp.tile([P, NSC, NH, D + 1], BF16, tag="v")
            attn_sb = pp.tile([P, PAIRS, S], BF16, tag="attn")

            nc.gpsimd.memset(k_ext, 0.0)
            nc.vector.memset(v_sb[:, :, :, D:D + 1], 1.0)

            # ---- Phase 1: QKV projections ----
            p1 = tc.tile_pool(name="qkvps", bufs=2, space="PSUM")
            qkps = vps = p1.__enter__()
            for i in range(NST):
                sl = slice(i * ST, (i + 1) * ST)
                xt = xtp.tile([P, NCC, ST], BF16, tag="xt")
                nc.scalar.dma_start(xt[:, 0:NCC // 2], xT.ap()[i][:, 0:NCC // 2])
                nc.scalar.dma_start(xt[:, NCC // 2:], xT.ap()[i][:, NCC // 2:])
                for m in range(2 * PAIRS * 2 // 2):  # 8 e' chunks
                    ps = qkps.tile([P, ST], F32, tag="ps")
                    for c in range(NCC):
                        nc.tensor.matmul(
                            ps, lhsT=wqk_sb[:, c, m * P:(m + 1) * P],
                            rhs=xt[:, c, :], start=(c == 0), stop=(c == NCC - 1))
                    pr = m // 2
                    if m % 2 == 0:  # Q pair
                        nc.scalar.activation(q_sb[:, pr, sl], ps, AF.Identity,
                                             bias=bqk_sb[:, m:m + 1])
                    else:  # K pair -> zero-extended slots
                        nc.scalar.activation(k_ext[0:D, 2 * pr, sl], ps[0:D],
                                             AF.Identity, bias=bqk_sb[0:D, m:m + 1])
                        nc.scalar.activation(k_ext[D:P, 2 * pr + 1, sl], ps[D:P],
                                             AF.Identity, bias=bqk_sb[D:P, m:m + 1])
                # V projection for this s-tile's 4 s-chunks
                for s4 in range(4):
                    sc = 4 * i + s4
                    vp = vps.tile([P, NH * D], F32, tag="ps")
                    for c in range(NCC):
                        nc.tensor.matmul(
                            vp, lhsT=xt[:, c, s4 * P:(s4 + 1) * P],
                            rhs=wv_sb[:, c, :], start=(c == 0), stop=(c == NCC - 1))
                    nc.vector.tensor_add(
                        out=v_sb[:, sc, :, 0:D],
                        in0=vp.rearrange("p (h d) -> p h d", h=NH),
                        in1=bv_bc.rearrange("p (h d) -> p h d", h=NH))

            # ---- Phase 2: attention (i-outer; outproj interleaved per i) ----
            p2a = tc.tile_pool(name="scps", bufs=2, space="PSUM")
            scps = p2a.__enter__()
            p2b = tc.tile_pool(name="avps", bufs=1, space="PSUM")
            avps = p2b.__enter__()
            for i in range(NST):
                qsl = slice(i * ST, (i + 1) * ST)
                for pr in range(PAIRS):
                    s0, s1 = 2 * pr, 2 * pr + 1
                    av0 = avps.tile([P, ST], F32, tag="av0")
                    av1 = avps.tile([P, ST], F32, tag="av1")
                    js = k_chunks_for_pair(pr, i)
                    for jj, j in enumerate(js):
                        # q columns below the diagonal of this k-chunk are
                        # fully masked; clip them (qlo is 0 for jj == 0).
                        qlo = max(0, P * j - ST * i)
                        w = ST - qlo
                        qs = slice(i * ST + qlo, (i + 1) * ST)
                        scp = scps.tile([P, 2 * ST], F32, tag="scp")
                        scp3 = scp.rearrange("p (t st) -> p t st", t=2)
                        nc.tensor.matmul(
                            scp[:, 0:w], lhsT=k_ext[:, s0, j * P:(j + 1) * P],
                            rhs=q_sb[:, pr, qs], start=True, stop=True)
                        nc.tensor.matmul(
                            scp[:, ST:ST + w], lhsT=k_ext[:, s1, j * P:(j + 1) * P],
                            rhs=q_sb[:, pr, qs], start=True, stop=True)
                        pm = pmp.tile([P, 2 * ST], BF16, tag="pm")
                        pm3 = pm.rearrange("p (t st) -> p t st", t=2)
                        nc.scalar.activation(pm3[:, :, 0:w], scp3[:, :, 0:w],
                                             AF.Exp)
                        pm2 = pmp.tile([P, 2 * ST], BF16, tag="pm2")
                        pm23 = pm2.rearrange("p (t st) -> p t st", t=2)
                        a0 = EMOFF - P * j + ST * i + qlo
                        nc.vector.tensor_mul(
                            pm23[:, :, 0:w], pm3[:, :, 0:w],
                            em_sb[:, s0:s0 + 2, a0:a0 + w])
                        nc.tensor.matmul(
                            av0[0:D + 1, qlo:ST], lhsT=v_sb[:, j, s0, :],
                            rhs=pm2[:, 0:w],
                            start=(jj == 0), stop=(jj == len(js) - 1))
                        nc.tensor.matmul(
                            av1[0:D + 1, qlo:ST], lhsT=v_sb[:, j, s1, :],
                            rhs=pm2[:, ST:ST + w],
                            start=(jj == 0), stop=(jj == len(js) - 1))
                    for odd, av in ((0, av0), (1, av1)):
                        lrow = misc.tile([D + 1, ST], F32, tag="lrow")
                        nc.vector.tensor_copy(lrow[D:D + 1, :], av[D:D + 1, :])
                        avs = misc.tile([D, ST], F32, tag="avs")
                        nc.scalar.activation(avs, av[0:D, :], AF.Copy)
                        scratch = drp.tile([1, ST], F32, tag="lscr")
                        nc.sync.dma_start(scratch, lrow[D:D + 1, :])
                        l_bc = misc.tile([D, ST], F32, tag="lbc")
                        nc.sync.dma_start(
                            l_bc,
                            bass.AP(tensor=scratch.tensor, offset=scratch.offset,
                                    ap=[[0, D]] + scratch.ap[1:]))
                        linv_bc = misc.tile([D, ST], F32, tag="linvbc")
                        nc.vector.reciprocal_approx_fast(out=linv_bc, in_=l_bc)
                        if not odd:
                            nc.vector.tensor_mul(attn_sb[0:D, pr, qsl],
                                                 avs, linv_bc)
                        else:
                            atmp = misc.tile([D, ST], BF16, tag="atmp")
                            nc.vector.tensor_mul(atmp, avs, linv_bc)
                            nc.sync.dma_start(attn_sb[D:P, pr, qsl], atmp)
                # output projection for this q-tile (reuses qkv psum banks)
                for qc in range(4 * i, 4 * i + 4):
                    for n in range(E // ST):
                        op = qkps.tile([P, ST], F32, tag="ps")
                        for p_ in range(PAIRS):
                            nc.tensor.matmul(
                                op, lhsT=attn_sb[:, p_, qc * P:(qc + 1) * P],
                                rhs=wout_sb[:, p_, n * ST:(n + 1) * ST],
                                start=(p_ == 0), stop=(p_ == PAIRS - 1))
                        ot = osb.tile([P, ST], F32, tag="ot")
                        nc.scalar.activation(ot, op, AF.Copy)
                        nc.sync.dma_start(
                            out.ap()[qc * P:(qc + 1) * P, n * ST:(n + 1) * ST], ot)
            p2b.__exit__(None, None, None)
            p2a.__exit__(None, None, None)
            p1.__exit__(None, None, None)

    nc.compile()
    _PROG_CACHE['prog'] = nc
    return nc


def _head_groups():
    return [SLOT_HEADS_A, SLOT_HEADS_B]


def _prep_core_inputs(x, W_qkv, b_qkv, W_out, b_out):
    """Build the 8 per-core input dicts (host-side shard + transform)."""
    groups = _head_groups()
    bf = ml_dtypes.bfloat16
    per_group = []
    for heads in groups:
        qcols = []
        kcols = []
        bqk_l = []
        for p in range(PAIRS):
            h0, h1 = heads[2 * p], heads[2 * p + 1]
            wq = np.concatenate([W_qkv[:, h0 * D:(h0 + 1) * D],
                                 W_qkv[:, h1 * D:(h1 + 1) * D]], axis=1) / 8.0
            wk = np.concatenate([W_qkv[:, E + h0 * D:E + (h0 + 1) * D],
                                 W_qkv[:, E + h1 * D:E + (h1 + 1) * D]], axis=1)
            qcols.append(wq)
            kcols.append(wk)
            bqk_l.append(np.concatenate([b_qkv[h0 * D:(h0 + 1) * D],
                                         b_qkv[h1 * D:(h1 + 1) * D]]) / 8.0)
            bqk_l.append(np.concatenate([b_qkv[E + h0 * D:E + (h0 + 1) * D],
                                         b_qkv[E + h1 * D:E + (h1 + 1) * D]]))
        # interleave Q-pair / K-pair chunks: m even = Q, m odd = K
        wqk_l = np.empty((E, 2 * NH * D), dtype=np.float32)
        for p in range(PAIRS):
            wqk_l[:, (2 * p) * P:(2 * p + 1) * P] = qcols[p]
            wqk_l[:, (2 * p + 1) * P:(2 * p + 2) * P] = kcols[p]
        bqk_full = np.empty(2 * NH * D, dtype=np.float32)
        for m in range(2 * PAIRS):
            bqk_full[m * P:(m + 1) * P] = bqk_l[m]
        wv_l = np.concatenate(
            [W_qkv[:, 2 * E + h * D:2 * E + (h + 1) * D] for h in heads], axis=1)
        bv_l = np.concatenate(
            [b_qkv[2 * E + h * D:2 * E + (h + 1) * D] for h in heads])
        wout_l = np.concatenate([W_out[h * D:(h + 1) * D, :] for h in heads],
                                axis=0)
        # pre-tile into the exact SBUF layouts for large-descriptor DMAs
        wqk_t = np.ascontiguousarray(
            wqk_l.reshape(NCC, P, 2 * NH * D).transpose(1, 0, 2)).astype(bf)
        wv_t = np.ascontiguousarray(
            wv_l.reshape(NCC, P, NH * D).transpose(1, 0, 2)).astype(bf)
        wout_t = np.ascontiguousarray(
            wout_l.reshape(PAIRS, P, E).transpose(1, 0, 2)).astype(bf)
        bqk_t = np.ascontiguousarray(
            bqk_full.reshape(NCC, P).T).astype(np.float32)
        per_group.append(dict(
            wqk=wqk_t, wv=wv_t, wout=wout_t,
            bqk=bqk_t, bv=bv_l.astype(bf),
            em=_build_em(heads)))

    in_maps = []
    xt_cache = {}
    for c in range(N_CORES):
        b, g = c // 2, c % 2
        m = dict(per_group[g])
        if b not in xt_cache:
            xt = x[b].T  # [E, S]
            xt_cache[b] = np.ascontiguousarray(
                xt.reshape(NCC, P, NST, ST).transpose(2, 1, 0, 3)).astype(bf)
        m['xT'] = xt_cache[b]
        in_maps.append(m)
    return in_maps


def _run(inputs, trace=False, tmpdir=None, trace_cores=None):
    x = np.asarray(inputs['x'], dtype=np.float32)
    W_qkv = np.asarray(inputs['W_qkv'], dtype=np.float32)
    b_qkv = np.asarray(inputs['b_qkv'], dtype=np.float32)
    W_out = np.asarray(inputs['W_out'], dtype=np.float32)
    b_out = np.asarray(inputs['b_out'], dtype=np.float32)

    nc = _build_program()
    in_maps = _prep_core_inputs(x, W_qkv, b_qkv, W_out, b_out)
    res = bass_utils.run_bass_kernel_spmd(
        nc, in_maps, core_ids=list(range(N_CORES)), trace=trace, tmpdir=tmpdir,
        trace_cores=trace_cores)
    out = np.empty((B, S, E), dtype=np.float32)
    for b in range(B):
        out[b] = res.results[2 * b]['out'] + res.results[2 * b + 1]['out'] + b_out
    return out, res


def kernel(**inputs) -> np.ndarray:
    out, _ = _run(inputs)
    return out
